# revision 10
# baseline (speedup 1.0000x reference)
"""Trainium2 Bass kernel for nn_LongTermMemoryMLP.

Per-batch-weight 3-layer MLP:
    h0 = relu(q @ W0^T + b0); h1 = relu(h0 @ W1^T + b1); out = h1 @ W2^T + b2
with q: [B,S,DIN], W0: [B,DH,DIN], W1: [B,DH,DH], W2: [B,DOUT,DH], B=8.

Sharding: data-parallel over batch — one batch sample (and its weight slabs)
per NeuronCore, 8 cores, no cross-core communication.

Device-side strategy: activations are kept feature-major ([feature, seq],
feature on partitions) so every layer is a plain accumulated matmul with the
(pre-transposed) weights as the stationary operand and the activations as the
moving operand — no on-chip transposes. The final layer flips orientation
(stationary = activation tile, moving = W2^T) so the output lands seq-major
and can be DMA'd out contiguously. Inputs are pre-transposed AND pre-cast to
bf16 on the host: bf16 streams at the PE's full 1 row/cycle (518 cycles
measured per 128x128x512 matmul, the warm roofline) and halves all input DMA
traffic, which bounds the startup ramp. Weights and each seq-chunk of the
query load as single ~0.5-1 MiB DMAs (small transfers run at <50% DMA
efficiency; ~1 MiB runs at ~80%). Accumulation stays fp32 in PSUM; measured
end-to-end relative error is ~4e-3 against the fp32 reference.
"""

import numpy as np

import ml_dtypes

import concourse.bass as bass
import concourse.tile as tile
from concourse import bacc, mybir
from concourse.bass_utils import run_bass_kernel_spmd

B, S, DIN, DH, DOUT = 8, 4096, 512, 1024, 512
SC = 512  # seq chunk processed per pipeline iteration

BF16 = mybir.dt.bfloat16
F32 = mybir.dt.float32


def build_nc():
    nc = bacc.Bacc("TRN2")
    qT = nc.dram_tensor("qT", (DIN, S), BF16, kind="ExternalInput")
    w0t = nc.dram_tensor("w0t", (DIN, DH), BF16, kind="ExternalInput")
    w1t = nc.dram_tensor("w1t", (DH, DH), BF16, kind="ExternalInput")
    w2t = nc.dram_tensor("w2t", (DH, DOUT), BF16, kind="ExternalInput")
    b0 = nc.dram_tensor("b0", (DH,), F32, kind="ExternalInput")
    b1 = nc.dram_tensor("b1", (DH,), F32, kind="ExternalInput")
    b2 = nc.dram_tensor("b2", (DOUT,), F32, kind="ExternalInput")
    out = nc.dram_tensor("out", (S, DOUT), BF16, kind="ExternalOutput")

    K0 = DIN // 128   # 4  k-tiles, layer 0
    K1 = DH // 128    # 8  k-tiles, layers 1/2
    M0 = DH // 128    # 8  m-tiles (feature tiles of h0/h1)
    MT = SC // 128    # 4  seq m-tiles per chunk, layer 2
    NCH = S // SC     # 8  chunks

    Relu = mybir.ActivationFunctionType.Relu

    with tile.TileContext(nc) as tc:
        with (
            tc.tile_pool(name="weights", bufs=1) as wpool,
            tc.tile_pool(name="biases", bufs=1) as bpool,
            tc.tile_pool(name="acts", bufs=2) as apool,
            tc.tile_pool(name="qin", bufs=2) as qpool,
            tc.tile_pool(name="outp", bufs=4) as opool,
            tc.tile_pool(name="psum0", bufs=2, space="PSUM") as ppool0,
            tc.tile_pool(name="psum1", bufs=3, space="PSUM") as ppool1,
            tc.tile_pool(name="psum2", bufs=3, space="PSUM") as ppool2,
        ):
            # Pre-warm the PE clock gate (HAM) with dummy matmuls on garbage
            # data while the startup DMAs land: the real matmul stream then
            # starts at 2.4 GHz.
            g_lhs = apool.tile([128, 128], BF16, tag="warm_lhs")
            g_rhs = apool.tile([128, SC], BF16, tag="warm_rhs")
            nc.vector.memset(g_lhs, 0.0)
            nc.vector.memset(g_rhs, 0.0)
            warm_ps = ppool0.tile([128, SC], F32, tag="ps0")
            N_WARM = 12
            for i in range(N_WARM):
                nc.tensor.matmul(
                    warm_ps, lhsT=g_lhs, rhs=g_rhs,
                    start=(i == 0), stop=(i == N_WARM - 1),
                )

            # Startup loads, spread over the three DMA-issuing engines
            # (sync/scalar HWDGE rings + gpsimd SWDGE), first-chunk operands
            # first. Each ring sustains only ~130-170 GB/s, so the
            # chunk-0-critical tensors (w0, q0) are split in half across
            # rings / issued as two pieces so layer 0 can start on the k<2
            # tiles while k>=2 are still in flight:
            #   sync:   q(c0) k<2 | q(c0) k>=2 | q(c1)   (then steady q)
            #   scalar: w0 k<2 | w1[k<4] | w2
            #   gpsimd: b0 b1 | w0 k>=2 | w1[k>=4] | b2
            # Weight k-tiles live as the middle dim of one 3D SBUF tile.
            w0a_sb = wpool.tile([128, K0 // 2, DH], BF16, tag="w0a")
            w0b_sb = wpool.tile([128, K0 // 2, DH], BF16, tag="w0b")
            b0_sb = bpool.tile([128, M0], F32, tag="b0")
            b1_sb = bpool.tile([128, M0], F32, tag="b1")
            nc.gpsimd.dma_start(out=b0_sb, in_=b0[:].rearrange("(m p) -> p m", p=128))
            nc.gpsimd.dma_start(out=b1_sb, in_=b1[:].rearrange("(m p) -> p m", p=128))
            nc.scalar.dma_start(
                out=w0a_sb,
                in_=w0t[0:DIN // 2, :].rearrange("(k p) h -> p k h", p=128),
            )
            nc.gpsimd.dma_start(
                out=w0b_sb,
                in_=w0t[DIN // 2:DIN, :].rearrange("(k p) h -> p k h", p=128),
            )

            def w0_slice(k, m):
                t = w0a_sb if k < K0 // 2 else w0b_sb
                return t[:, k % (K0 // 2), m * 128:(m + 1) * 128]

            def load_q(c, split=False):
                s0 = c * SC
                t = qpool.tile([128, K0, SC], BF16, tag="q", name=f"q{c}")
                if split:
                    nc.sync.dma_start(
                        out=t[:, 0:K0 // 2, :],
                        in_=qT[0:DIN // 2, s0:s0 + SC].rearrange(
                            "(k p) s -> p k s", p=128),
                    )
                    nc.sync.dma_start(
                        out=t[:, K0 // 2:K0, :],
                        in_=qT[DIN // 2:DIN, s0:s0 + SC].rearrange(
                            "(k p) s -> p k s", p=128),
                    )
                else:
                    nc.sync.dma_start(
                        out=t,
                        in_=qT[:, s0:s0 + SC].rearrange("(k p) s -> p k s", p=128),
                    )
                return t

            q0_sb = load_q(0, split=True)
            q1_sb = load_q(1)

            w1a_sb = wpool.tile([128, K1 // 2, DH], BF16, tag="w1a")
            w1b_sb = wpool.tile([128, K1 // 2, DH], BF16, tag="w1b")
            nc.scalar.dma_start(
                out=w1a_sb, in_=w1t[0:DH // 2, :].rearrange("(k p) h -> p k h", p=128)
            )
            nc.gpsimd.dma_start(
                out=w1b_sb, in_=w1t[DH // 2:DH, :].rearrange("(k p) h -> p k h", p=128)
            )

            def w1_slice(k, m):
                t = w1a_sb if k < K1 // 2 else w1b_sb
                return t[:, k % (K1 // 2), m * 128:(m + 1) * 128]

            w2_sb = wpool.tile([128, K1, DOUT], BF16, tag="w2")
            nc.scalar.dma_start(
                out=w2_sb, in_=w2t[:, :].rearrange("(k p) o -> p k o", p=128)
            )
            b2_sb = bpool.tile([128, DOUT], F32, tag="b2")
            b2_ap = b2[:]
            b2_bcast = bass.AP(
                tensor=b2_ap.tensor,
                offset=b2_ap.offset,
                ap=[[0, 128]] + [list(d) for d in b2_ap.ap],
            )
            nc.gpsimd.dma_start(out=b2_sb, in_=b2_bcast)

            def layer0(c, q_sb):
                h0_sb = []
                for m in range(M0):
                    ps = ppool0.tile([128, SC], F32, tag="ps0", name=f"ps0_{c}_{m}")
                    for k in range(K0):
                        nc.tensor.matmul(
                            ps,
                            lhsT=w0_slice(k, m),
                            rhs=q_sb[:, k, :],
                            start=(k == 0),
                            stop=(k == K0 - 1),
                        )
                    h = apool.tile([128, SC], BF16, tag=f"h0_{m}", name=f"h0_{c}_{m}")
                    nc.scalar.activation(h, ps, Relu, bias=b0_sb[:, m:m + 1])
                    h0_sb.append(h)
                return h0_sb

            def layers12(c, h0_sb):
                s0 = c * SC
                last = c == NCH - 1
                h1_sb = []
                for m in range(M0):
                    ps = ppool1.tile([128, SC], F32, tag="ps1", name=f"ps1_{c}_{m}")
                    for k in range(K1):
                        nc.tensor.matmul(
                            ps,
                            lhsT=w1_slice(k, m),
                            rhs=h0_sb[k],
                            start=(k == 0),
                            stop=(k == K1 - 1),
                        )
                    h = apool.tile([128, SC], BF16, tag=f"h1_{m}", name=f"h1_{c}_{m}")
                    nc.scalar.activation(h, ps, Relu, bias=b1_sb[:, m:m + 1])
                    h1_sb.append(h)

                for mt in range(MT):
                    ps = ppool2.tile([128, DOUT], F32, tag="ps2", name=f"ps2_{c}_{mt}")
                    for k in range(K1):
                        nc.tensor.matmul(
                            ps,
                            lhsT=h1_sb[k][:, mt * 128:(mt + 1) * 128],
                            rhs=w2_sb[:, k, :],
                            start=(k == 0),
                            stop=(k == K1 - 1),
                        )
                    ot = opool.tile([128, DOUT], BF16, tag="ot", name=f"ot_{c}_{mt}")
                    r0 = s0 + mt * 128
                    if last and mt == MT - 1:
                        # Tail trim: halve the strictly-serial PSUM->add->DMA
                        # chain after the very last matmul.
                        H = DOUT // 2
                        nc.vector.tensor_add(ot[:, 0:H], ps[:, 0:H], b2_sb[:, 0:H])
                        nc.scalar.dma_start(
                            out=out[r0:r0 + 128, 0:H], in_=ot[:, 0:H]
                        )
                        nc.vector.tensor_add(ot[:, H:], ps[:, H:], b2_sb[:, H:])
                        nc.sync.dma_start(out=out[r0:r0 + 128, H:], in_=ot[:, H:])
                    else:
                        nc.vector.tensor_add(ot, ps, b2_sb)
                        eng = nc.scalar if mt % 2 == 0 else nc.sync
                        eng.dma_start(out=out[r0:r0 + 128, :], in_=ot)

            # Software pipeline: emit L0 of chunk c+1 ahead of L1/L2 of
            # chunk c, so the matmul stream never depends on a DMA issued
            # less than a full chunk earlier.
            h0_cur = layer0(0, q0_sb)
            for c in range(NCH):
                h0_next = None
                if c + 1 < NCH:
                    q_sb = q1_sb if c + 1 == 1 else load_q(c + 1)
                    h0_next = layer0(c + 1, q_sb)
                layers12(c, h0_cur)
                h0_cur = h0_next
    nc.finalize()
    return nc


_NC = None


def _get_nc():
    global _NC
    if _NC is None:
        _NC = build_nc()
    return _NC


def make_in_maps(inputs):
    bf16 = ml_dtypes.bfloat16
    q, W0, b0, W1, b1, W2, b2 = (
        inputs["query"], inputs["W0"], inputs["b0"], inputs["W1"],
        inputs["b1"], inputs["W2"], inputs["b2"],
    )
    in_maps = []
    for b in range(B):
        in_maps.append({
            "qT": np.ascontiguousarray(np.asarray(q[b]).T.astype(bf16)),
            "w0t": np.ascontiguousarray(np.asarray(W0[b]).T.astype(bf16)),
            "w1t": np.ascontiguousarray(np.asarray(W1[b]).T.astype(bf16)),
            "w2t": np.ascontiguousarray(np.asarray(W2[b]).T.astype(bf16)),
            "b0": np.asarray(b0[b], dtype=np.float32),
            "b1": np.asarray(b1[b], dtype=np.float32),
            "b2": np.asarray(b2[b], dtype=np.float32),
        })
    return in_maps


def run(inputs, trace=False):
    nc = _get_nc()
    in_maps = make_in_maps(inputs)
    res = run_bass_kernel_spmd(nc, in_maps, core_ids=list(range(B)), trace=trace)
    out = np.stack(
        [np.asarray(r["out"]).astype(np.float32) for r in res.results]
    )
    return out, res


def kernel(**inputs) -> np.ndarray:
    out, _ = run(inputs, trace=False)
    return out
